# revision 1
# baseline (speedup 1.0000x reference)
"""AdaptiveGCN forward on 8 TRN2 NeuronCores (Bass/Tile).

Math (per the nn.Module reference):
  xr  = permute/reshape of x into (B*L, C, N)      [torch-faithful raw reshape]
  adp = softmax(relu(nodevec1 @ nodevec2), -1)
  out_list = [xr] + [xr@a^T, xr@a^T@a^T  for a in (a1, a2, adp)]
  o   = w @ concat(out_list, channel axis) + b     (1x1 conv)
  return o.reshape(B, L, N, C)                     [raw reshape]

Distribution: pure data-parallel over B (8 cores, 1 batch row each),
weights replicated, no collectives in forward.

Key layout fact (derived + numerically verified): per batch b the reference's
xr rows [b*L, (b+1)*L) are exactly  x[b].reshape(64, 65536).T.reshape(64, C, N).
Per output row m, T := xr[m].T (node-major, (N, C)) is reachable from the
contiguous slice x[b][:, 8m:8m+8, :] by partition-preserving strided copies:
  T[u_hi*64 + n_lo, k*128 + c_hi*16 + c_lo] = x[b][n_lo, 8m + c_hi, c_lo*8 + 2k + u_hi]
(the x slice is DMAed into both partition halves so the u_hi=1 copy stays
lane-local).

Order-2 diffusion uses (a^T)^2 = (a@a)^T so every concat member is a single
matmul from X: Y_j[m] = X_m @ P_j with P in {I, a1^T, (a1^2)^T, a2^T, (a2^2)^T,
adp^T, (adp^2)^T}. With lhsT = T-chunks (node-major) the PE emits Y_j[m]
channel-major in PSUM, which directly feeds the 1x1-conv matmuls
(lhsT = w^T chunks), accumulating all 7 concat members into one PSUM bank.
Y_0 (the identity member) is produced by PE transpose-mode (cheap N=128
passes) instead of a full identity matmul.

Startup choreography matters (~25 us at stake): concurrent DMA queues share
the 16 SDMA engines round-robin, so the weight load is explicitly sequenced
before the x-prefetch chain (add_dep_helper), weights arrive pre-arranged
from the host so every DMA is contiguous, and the first two m-groups are
half-sized so the PE pipeline ramps sooner.
"""

import numpy as np

import concourse.bass as bass
import concourse.bacc as bacc
import concourse.mybir as mybir
import concourse.tile as tile
from concourse.bass_utils import run_bass_kernel_spmd
from concourse.tile_rust import add_dep_helper

F32 = mybir.dt.float32
BF16 = mybir.dt.bfloat16

B, L, N, C = 8, 64, 512, 128
NK = N // 128          # 4 contraction chunks of 128
NJ = 7                 # concat members
AF = mybir.ActivationFunctionType

# m-groups: (first m, count); small leading groups ramp the pipeline faster
MGROUPS = [(0, 1), (1, 1), (2, 1), (3, 1)] + [(4 + 4 * i, 4) for i in range(15)]

_CACHE = {}


def build_graph():
    nc = bacc.Bacc("TRN2", target_bir_lowering=False, debug=False, num_devices=8)

    x_d = nc.declare_dram_parameter("x", [L, N, C], F32, isOutput=False)
    # nvs[p, w*512 + v]: w=0 -> nodevec1.T, w=1 -> nodevec2
    nvs_d = nc.declare_dram_parameter("nvs", [10, 2 * N], mybir.dt.float32r, isOutput=False)
    # wts[p, (w*4 + k)*512 + v] = M_w[128k + p, v], M = [a1^T, a2^T, a1, a2]
    wts_d = nc.declare_dram_parameter("wts", [128, 4 * NK * N], BF16, isOutput=False)
    # wtc[c, j*128 + o] = w[o, j*128 + c]
    wtc_d = nc.declare_dram_parameter("wtc", [C, NJ * C], BF16, isOutput=False)
    b_d = nc.declare_dram_parameter("bias", [C, 1], F32, isOutput=False)
    out_d = nc.declare_dram_parameter("out", [L, C, N], F32, isOutput=True)

    with tile.TileContext(nc) as tc:
        with (
            tc.tile_pool(name="const", bufs=1) as const,
            tc.tile_pool(name="setup", bufs=1) as setup,
            tc.tile_pool(name="smax", bufs=2) as smax,
            tc.tile_pool(name="sbig", bufs=3) as sbig_pool,
            tc.tile_pool(name="tcat", bufs=8) as tcat_pool,
            tc.tile_pool(name="ysb", bufs=24) as ysb_pool,
            tc.tile_pool(name="outsb", bufs=4) as outsb_pool,
            tc.tile_pool(name="ypsum", bufs=5, space=bass.MemorySpace.PSUM) as ypsum_pool,
            tc.tile_pool(name="y0psum", bufs=1, space=bass.MemorySpace.PSUM) as y0psum_pool,
            tc.tile_pool(name="opsum", bufs=2, space=bass.MemorySpace.PSUM) as opsum_pool,
        ):
            # ---------------- PE warm-up ------------------------------------
            # Dep-free dummy matmuls: they run during the otherwise-idle
            # window while the first DMAs land, and hold the HAM activity
            # window busy so the real stream starts at 2.4 GHz. Inputs are
            # uninitialized SBUF (garbage), output goes to a scratch PSUM
            # bank that is recycled afterwards.
            warm_in = setup.tile([128, N], BF16, tag="warm")
            nc.gpsimd.memset(warm_in[:], 0.0)
            warm_ps = opsum_pool.tile([128, N], F32, tag="op", name="warm_ps")
            for _ in range(14):
                nc.tensor.matmul(warm_ps[:], warm_in[:, 0:128], warm_in[:],
                                 start=True, stop=True)

            # ---------------- weights (contiguous, pre-arranged on host) ----
            nvs_sb = setup.tile([10, 2 * N], mybir.dt.float32r, tag="nvs")
            nc.sync.dma_start(out=nvs_sb[:], in_=nvs_d[:])
            nv1t_sb = nvs_sb[:, 0:N]
            nv2_sb = nvs_sb[:, N:2 * N]

            wts_sb = const.tile([128, 4 * NK * N], BF16, tag="wts")
            HW = 2 * NK * N
            nc.sync.dma_start(out=wts_sb[:, 0:HW], in_=wts_d[0:128, 0:HW])
            wts_dma = nc.scalar.dma_start(out=wts_sb[:, HW:2 * HW],
                                          in_=wts_d[0:128, HW:2 * HW])
            wt_sb = const.tile([C, NJ * C], BF16, tag="wt")
            nc.scalar.dma_start(out=wt_sb[:], in_=wtc_d[:])
            b_sb = const.tile([C, 1], F32, tag="bsb")
            nc.scalar.dma_start(out=b_sb[:], in_=b_d[:])

            p_sb = {j: const.tile([128, NK * N], BF16, tag=f"p{j}", name=f"p{j}")
                    for j in (2, 4, 5, 6)}
            p_sb[1] = wts_sb[:, 0:NK * N]
            p_sb[3] = wts_sb[:, NK * N:2 * NK * N]
            a1n = wts_sb[:, 2 * NK * N:3 * NK * N]
            a2n = wts_sb[:, 3 * NK * N:4 * NK * N]
            adpn = setup.tile([128, NK * N], BF16, tag="adpn")

            i128 = const.tile([128, 128], BF16, tag="i128")
            nc.gpsimd.memset(i128[:], 0.0)
            nc.gpsimd.affine_select(
                out=i128[:], in_=i128[:],
                compare_op=mybir.AluOpType.not_equal, fill=1.0,
                base=0, pattern=[[-1, 128]], channel_multiplier=1,
            )

            # ---------------- adaptive adjacency (softmax chain) ------------
            # relu(E) >= 0 and |E| <~ 15, so exp never overflows in f32 and
            # the max-subtraction of a stable softmax can be skipped.
            for r in range(NK):
                ep = ypsum_pool.tile([128, N], F32, tag="yp", name="ep")
                nc.tensor.matmul(ep[:], nv1t_sb[:, 128 * r:128 * (r + 1)], nv2_sb[:],
                                 start=True, stop=True)
                es = smax.tile([128, N], F32, tag="es")
                nc.scalar.activation(es[:], ep[:], AF.Relu)
                pex = smax.tile([128, N], F32, tag="pex")
                sm = smax.tile([128, 1], F32, tag="sm")
                nc.scalar.activation(pex[:], es[:], AF.Exp, accum_out=sm[:])
                rs = smax.tile([128, 1], F32, tag="rs")
                nc.vector.reciprocal(rs[:], sm[:])
                nc.vector.tensor_scalar_mul(adpn[:, r * N:(r + 1) * N], pex[:], rs[:])

            def square(nat, src_j, dst_j):
                # P_dst = P_src @ P_src, lhsT = natural-orientation chunks
                for r in range(NK):
                    pp = opsum_pool.tile([128, N], F32, tag="op", name="pps")
                    for k in range(NK):
                        nc.tensor.matmul(
                            pp[:],
                            nat[:, k * N + 128 * r:k * N + 128 * (r + 1)],
                            p_sb[src_j][:, k * N:(k + 1) * N],
                            start=(k == 0), stop=(k == NK - 1))
                    nc.scalar.copy(p_sb[dst_j][:, r * N:(r + 1) * N], pp[:])

            # wts-dependent squares first: they fill the PE while the ACT
            # softmax chain runs; then the adp-dependent P5/P6
            square(a1n, 1, 2)
            square(a2n, 3, 4)

            # P5 = adp^T via PE transpose-mode (needs only adp)
            for r in range(NK):
                pp = y0psum_pool.tile([128, N], BF16, tag="y0p", name="pp5")
                for k in range(NK):
                    nc.tensor.matmul(
                        pp[:, 128 * k:128 * (k + 1)],
                        adpn[:, k * N + 128 * r:k * N + 128 * (r + 1)],
                        i128[:], is_transpose=True,
                        start=(k == 0), stop=(k == NK - 1))
                nc.scalar.copy(p_sb[5][:, r * N:(r + 1) * N], pp[:])

            square(adpn, 5, 6)

            # ---------------- main loop -------------------------------------
            prev_dma = wts_dma

            def load_group(m0, cnt):
                nonlocal prev_dma
                sb = sbig_pool.tile([128, cnt * 1024], F32, tag="sb", name="sb")
                src = x_d[:, 8 * m0:8 * (m0 + cnt), :].rearrange("a b c -> a (b c)")
                # duplicate into both partition halves (copies are lane-local);
                # chain on the previous load so concurrent DMA queues don't
                # round-robin-starve each other
                d1 = nc.sync.dma_start(out=sb[0:64, :], in_=src)
                d2 = nc.sync.dma_start(out=sb[64:128, :], in_=src)
                add_dep_helper(d1.ins, prev_dma.ins, sync=True,
                               reason="sequence x prefetch behind prior DMA")
                prev_dma = d2
                return sb

            def make_tcat(sb, t):
                tcat = tcat_pool.tile([128, N], BF16, tag="tc", name="tcat")
                smv = sb[:, t * 1024:(t + 1) * 1024].rearrange(
                    "p (ch cl nh) -> p nh ch cl", ch=8, cl=16, nh=8)
                outv = tcat.rearrange("p (k ch cl) -> p k ch cl", k=NK, ch=8, cl=16)
                nc.vector.tensor_copy(outv[0:64], smv[0:64, 0::2])
                nc.vector.tensor_copy(outv[64:128], smv[64:128, 1::2])
                return tcat

            def diffuse(tcat, j):
                # Y_j[m] channel-major into SBUF (bf16)
                ysb = ysb_pool.tile([128, N], BF16, tag="ys", name="ysb")
                if j == 0:
                    y0p = y0psum_pool.tile([128, N], BF16, tag="y0p", name="y0p")
                    for k in range(NK):
                        nc.tensor.matmul(
                            y0p[:, 128 * k:128 * (k + 1)],
                            tcat[:, 128 * k:128 * (k + 1)],
                            i128[:], is_transpose=True,
                            start=(k == 0), stop=(k == NK - 1))
                    nc.scalar.copy(ysb[:], y0p[:])
                else:
                    yp = ypsum_pool.tile([128, N], F32, tag="yp", name="yp")
                    for k in range(NK):
                        nc.tensor.matmul(yp[:], tcat[:, 128 * k:128 * (k + 1)],
                                         p_sb[j][:, k * N:(k + 1) * N],
                                         start=(k == 0), stop=(k == NK - 1))
                    if j % 2 == 0:
                        nc.scalar.copy(ysb[:], yp[:])
                    else:
                        nc.vector.tensor_copy(ysb[:], yp[:])
                return ysb

            def conv_store(m, y_sb):
                op = opsum_pool.tile([C, N], F32, tag="op", name="op")
                for j in range(NJ):
                    nc.tensor.matmul(op[:], wt_sb[:, C * j:C * (j + 1)], y_sb[j][:],
                                     start=(j == 0), stop=(j == NJ - 1))
                out_tile = outsb_pool.tile([C, N], F32, tag="ot", name="ot")
                nc.scalar.activation(out_tile[:], op[:],
                                     AF.Identity, bias=b_sb[:], scale=1.0)
                nc.scalar.dma_start(out=out_d[m, :, :], in_=out_tile[:])

            for (m0, cnt) in MGROUPS:
                sb = load_group(m0, cnt)
                for t in range(cnt):
                    tcat = make_tcat(sb, t)
                    y_sb = [diffuse(tcat, j) for j in range(NJ)]
                    conv_store(m0 + t, y_sb)

    nc.compile()
    return nc


def _get_compiled():
    if "nc" not in _CACHE:
        _CACHE["nc"] = build_graph()
    return _CACHE["nc"]


def make_in_maps(x, nodevec1, nodevec2, a1, a2, w, b):
    import ml_dtypes
    f32 = lambda a: np.asarray(a, dtype=np.float32)
    bf = lambda a: np.asarray(a, dtype=np.float32).astype(ml_dtypes.bfloat16)

    nvs = np.stack([f32(nodevec1).T, f32(nodevec2)], axis=1)       # (10, 2, 512)
    # wts[p, w, k, v] = M_w[128k + p, v], M = [a1^T, a2^T, a1, a2]
    ms = np.stack([bf(a1).T, bf(a2).T, bf(a1), bf(a2)], axis=0)    # (4, 512, 512)
    wts = ms.reshape(4, NK, 128, N).transpose(2, 0, 1, 3)          # (128, 4, 4, 512)
    # wtc[c, j, o] = w[o, j*128 + c]
    wtc = bf(w).reshape(C, NJ, C).transpose(2, 1, 0)               # (c, j, o)

    shared = {
        "nvs": np.ascontiguousarray(nvs).reshape(10, 2 * N),
        "wts": np.ascontiguousarray(wts).reshape(128, 4 * NK * N),
        "wtc": np.ascontiguousarray(wtc).reshape(C, NJ * C),
        "bias": np.ascontiguousarray(f32(b).reshape(C, 1)),
    }
    xs = f32(x)
    return [dict(shared, x=np.ascontiguousarray(xs[i])) for i in range(B)]


def kernel(x, nodevec1, nodevec2, a1, a2, w, b):
    nc = _get_compiled()
    in_maps = make_in_maps(x, nodevec1, nodevec2, a1, a2, w, b)
    res = run_bass_kernel_spmd(nc, in_maps, core_ids=list(range(B))).results
    out = np.concatenate([res[i]["out"] for i in range(B)], axis=0)  # (B*L, C, N)
    return out.reshape(B, L, N, C).astype(np.float32)

